# revision 26
# baseline (speedup 1.0000x reference)
"""Trainium2 Bass kernel for Conv1d_NN (KNN gather + conv) — 8-core SPMD.

Problem: x [16, 64, 2048] -> per batch: 3-NN by L2 distance over the 2048
columns, gather neighbor columns, contract with W [64, 64, 3] + bias.

Sharding: batch dim across 8 cores (2 batches/core), no cross-core comm.

Per-core pipeline (per batch):
  - Host passes x, mn = -0.5*sum(x^2, axis=c), wt = W transposed to [k][c,o].
  - s'[n, j] = x_n . x_j - |x_j|^2/2 via one fused matmul:
      lhsT = [x_tile; ones] (65 x 128), rhs = [x; mn] (65 x 2048)
    argmax_j s' == argmin_j dist^2 (row-constant |x_n|^2 dropped; scale 1/2).
  - Hardware top-8 per row (DVE max + max_index) -> top-3 neighbor indices.
  - Indices bounce through DRAM into the GPSIMD wrapped layout; one
    ap_gather pulls neigh [c, k*2048] columns of x.
  - Conv: 3 accumulating matmuls (lhsT = W_k^T [c, o]) -> psum [o, 2048],
    bias fused into the ACT psum->sbuf copy, DMA out.
"""

import sys

sys.path.insert(0, "/opt/trn_rl_repo")

from contextlib import ExitStack

import numpy as np

import concourse.bass as bass
import concourse.mybir as mybir
import concourse.tile as tile
from concourse import bacc
from concourse.bass_utils import run_bass_kernel_spmd

B, C, N, K = 16, 64, 2048, 3
O = 64
NCORES = 8
BPC = B // NCORES  # batches per core
NT = N // 128  # row tiles per batch
FCH = 512  # matmul free-dim chunk (one PSUM bank)

f32 = mybir.dt.float32
f32r = mybir.dt.float32r
i16 = mybir.dt.int16
u16 = mybir.dt.uint16

# distance matmul dtype. float32r (full-rate PE) was tested and REJECTED:
# its ~1.8e-4 relative error swaps neighbors (min 3rd-vs-4th gap on the
# graded input is 7.6e-5 absolute); fp32 matches the fp32 reference.
DIST_DT = f32


def build(dist_dt=DIST_DT, reps=1):
    nc = bacc.Bacc("TRN2", debug=False)
    # x2/mn/ones carry the distance-matmul dtype (f32r needs typed producers
    # end-to-end for the walrus verifier; np layout is identical to f32).
    x2 = nc.dram_tensor("x2", [BPC, C, N], dist_dt, kind="ExternalInput").ap()
    mn = nc.dram_tensor("mn", [BPC, 1, N], dist_dt, kind="ExternalInput").ap()
    ones = nc.dram_tensor("ones", [1, N], dist_dt, kind="ExternalInput").ap()
    wt = nc.dram_tensor("wt", [K, C, O], f32, kind="ExternalInput").ap()
    bias = nc.dram_tensor("bias", [O, 1], f32, kind="ExternalInput").ap()
    out = nc.dram_tensor("out", [BPC, O, N], f32, kind="ExternalOutput").ap()

    with tile.TileContext(nc) as tc, ExitStack() as ctx:
        const_p = ctx.enter_context(tc.tile_pool(name="const", bufs=1))
        xm_p = ctx.enter_context(tc.tile_pool(name="xm", bufs=2))
        xs_p = ctx.enter_context(tc.tile_pool(name="xs", bufs=2))
        sp_p = ctx.enter_context(tc.tile_pool(name="spsum", bufs=4, space="PSUM"))
        ssb_p = ctx.enter_context(tc.tile_pool(name="ssb", bufs=2))
        m8_p = ctx.enter_context(tc.tile_pool(name="m8", bufs=2))
        i8_p = ctx.enter_context(tc.tile_pool(name="i8", bufs=2))
        stg_p = ctx.enter_context(tc.tile_pool(name="stg", bufs=2))
        stgd_p = ctx.enter_context(tc.tile_pool(name="stgd", bufs=2, space="DRAM"))
        ng_p = ctx.enter_context(tc.tile_pool(name="neigh", bufs=2))
        cp_p = ctx.enter_context(tc.tile_pool(name="cpsum", bufs=2, space="PSUM"))
        osb_p = ctx.enter_context(tc.tile_pool(name="osb", bufs=2))

        wt_sb = const_p.tile([C, K * O], f32)
        nc.sync.dma_start(out=wt_sb[:], in_=wt.rearrange("k c o -> c k o"))
        bias_sb = const_p.tile([O, 1], f32)
        nc.sync.dma_start(out=bias_sb[:], in_=bias)

        for i in [i for _ in range(reps) for i in range(BPC)]:
            xm = xm_p.tile([C + 1, N], dist_dt)  # rows 0..63 = x, row 64 = mn
            nc.sync.dma_start(out=xm[0:C, :], in_=x2[i])
            nc.sync.dma_start(out=xm[C : C + 1, :], in_=mn[i])
            xs = xs_p.tile([C + 1, N], dist_dt)  # rows 0..63 = x, row 64 = ones
            nc.sync.dma_start(out=xs[0:C, :], in_=x2[i])
            nc.sync.dma_start(out=xs[C : C + 1, :], in_=ones)

            ob = osb_p.tile([O, N], f32)
            for t in range(NT):
                lhsT = xs[:, t * 128 : (t + 1) * 128]
                s_sb = ssb_p.tile([128, N], f32)
                # one PSUM tile per bank-sized chunk: deps stay per-chunk, so
                # ACT drains chunk c4 while the PE fills chunk c4+1.
                for c4 in range(N // FCH):
                    s = sp_p.tile([128, FCH], f32, tag="schunk")
                    nc.tensor.matmul(
                        s[:],
                        lhsT=lhsT,
                        rhs=xm[:, c4 * FCH : (c4 + 1) * FCH],
                        start=True,
                        stop=True,
                    )
                    nc.scalar.copy(
                        s_sb[:, c4 * FCH : (c4 + 1) * FCH],
                        s[:],
                    )
                m8 = m8_p.tile([128, 8], f32)
                nc.vector.max(m8[:], s_sb[:])
                i8 = i8_p.tile([128, 8], u16)
                nc.vector.max_index(i8[:], m8[:], s_sb[:])

                # this tile's 128x3 indices -> DRAM in the gather's wrapped
                # layout for a 384-index list (i = k*128 + r):
                # [p = r%16, slot = k*8 + r//16]
                sd = stgd_p.tile([16, K * 8], i16)
                sd_w = sd[:].rearrange("p (k a) -> a p k", k=K, a=8)
                nc.sync.dma_start(out=sd_w, in_=i8[:, 0:K].bitcast(i16))
                stg = stg_p.tile([C, K * 8], i16)
                # one DMA, 0-stride source dim replicates into all 4 Q7
                # core groups (each gpsimd core reads its own 16 partitions)
                nc.sync.dma_start(
                    out=stg[:],
                    in_=sd[:].rearrange("p f -> () p f").to_broadcast([4, 16, K * 8]),
                )

                # gather neighbors for these 128 columns (k-major), then the
                # 3-tap conv accumulates into one small psum chunk.
                ng = ng_p.tile([C, K * 128], f32)
                nc.gpsimd.ap_gather(
                    ng[:],
                    xm[0:C, :].bitcast(f32),
                    stg[:],
                    channels=C,
                    num_elems=N,
                    d=1,
                    num_idxs=K * 128,
                )
                cp = cp_p.tile([O, 128], f32)
                for k in range(K):
                    nc.tensor.matmul(
                        cp[:],
                        lhsT=wt_sb[:, k * O : (k + 1) * O],
                        rhs=ng[:, k * 128 : (k + 1) * 128],
                        start=(k == 0),
                        stop=(k == K - 1),
                    )
                nc.scalar.activation(
                    ob[:, t * 128 : (t + 1) * 128],
                    cp[:],
                    mybir.ActivationFunctionType.Identity,
                    bias=bias_sb[:],
                )
            nc.sync.dma_start(out=out[i], in_=ob[:])

    nc.compile()
    return nc


_NC_CACHE: dict = {}


def _get_nc():
    if "nc" not in _NC_CACHE:
        _NC_CACHE["nc"] = build()
    return _NC_CACHE["nc"]


def make_in_maps(x, W, b):
    x = np.ascontiguousarray(np.asarray(x, dtype=np.float32))
    W = np.ascontiguousarray(np.asarray(W, dtype=np.float32))
    b = np.ascontiguousarray(np.asarray(b, dtype=np.float32))
    mn_full = -0.5 * np.einsum("bcn,bcn->bn", x, x)  # [B, N]
    wt = np.ascontiguousarray(np.transpose(W, (2, 1, 0)))  # [K, C, O]
    bias = np.ascontiguousarray(b.reshape(O, 1))
    ones = np.ones((1, N), dtype=np.float32)
    in_maps = []
    for c in range(NCORES):
        sl = slice(c * BPC, (c + 1) * BPC)
        in_maps.append(
            {
                "x2": np.ascontiguousarray(x[sl]),
                "mn": np.ascontiguousarray(mn_full[sl][:, None, :]),
                "ones": ones,
                "wt": wt,
                "bias": bias,
            }
        )
    return in_maps


def kernel(x, W, b, _trace=False):
    nc = _get_nc()
    in_maps = make_in_maps(x, W, b)
    try:
        res = run_bass_kernel_spmd(nc, in_maps, list(range(NCORES)), trace=_trace)
    except ModuleNotFoundError:
        res = run_bass_kernel_spmd(nc, in_maps, list(range(NCORES)))
    outs = [res.results[c]["out"] for c in range(NCORES)]
    full = np.concatenate(outs, axis=0).astype(np.float32)
    if _trace:
        return full, res
    return full


# revision 30
# speedup vs baseline: 1321.4287x; 1321.4287x over previous
"""Trainium2 Bass kernel for Conv1d_NN (KNN gather + conv) — 8-core SPMD.

Problem: x [16, 64, 2048] -> per batch: 3-NN by L2 distance over the 2048
columns, gather neighbor columns, contract with W [64, 64, 3] + bias.

Sharding: batch dim across 8 cores (2 batches/core), no cross-core comm.

Per-core pipeline (per batch):
  - Host passes x, mn = -0.5*sum(x^2, axis=c), wt = W transposed to [k][c,o].
  - s'[n, j] = x_n . x_j - |x_j|^2/2 via one fused matmul:
      lhsT = [x_tile; ones] (65 x 128), rhs = [x; mn] (65 x 2048)
    argmax_j s' == argmin_j dist^2 (row-constant |x_n|^2 dropped; scale 1/2).
  - Hardware top-8 per row (DVE max + max_index) -> top-3 neighbor indices.
  - Indices bounce through DRAM into the GPSIMD wrapped layout; one
    ap_gather pulls neigh [c, k*2048] columns of x.
  - Conv: 3 accumulating matmuls (lhsT = W_k^T [c, o]) -> psum [o, 2048],
    bias fused into the ACT psum->sbuf copy, DMA out.
"""

import sys

sys.path.insert(0, "/opt/trn_rl_repo")

from contextlib import ExitStack

import numpy as np

import concourse.bass as bass
import concourse.mybir as mybir
import concourse.tile as tile
from concourse import bacc
from concourse.bass_utils import run_bass_kernel_spmd

B, C, N, K = 16, 64, 2048, 3
O = 64
NCORES = 8
BPC = B // NCORES  # batches per core
NT = N // 128  # row tiles per batch
FCH = 512  # matmul free-dim chunk (one PSUM bank)

f32 = mybir.dt.float32
f32r = mybir.dt.float32r
i16 = mybir.dt.int16
u16 = mybir.dt.uint16

# distance matmul dtype. float32r (full-rate PE) was tested and REJECTED:
# its ~1.8e-4 relative error swaps neighbors (min 3rd-vs-4th gap on the
# graded input is 7.6e-5 absolute); fp32 matches the fp32 reference.
DIST_DT = f32


def build(dist_dt=DIST_DT, reps=1):
    nc = bacc.Bacc("TRN2", debug=False)
    # x2/mn/ones carry the distance-matmul dtype (f32r needs typed producers
    # end-to-end for the walrus verifier; np layout is identical to f32).
    x2 = nc.dram_tensor("x2", [BPC, C, N], dist_dt, kind="ExternalInput").ap()
    mn = nc.dram_tensor("mn", [BPC, 1, N], dist_dt, kind="ExternalInput").ap()
    ones = nc.dram_tensor("ones", [1, N], dist_dt, kind="ExternalInput").ap()
    wt = nc.dram_tensor("wt", [K, C, O], f32, kind="ExternalInput").ap()
    bias = nc.dram_tensor("bias", [O, 1], f32, kind="ExternalInput").ap()
    out = nc.dram_tensor("out", [BPC, O, N], f32, kind="ExternalOutput").ap()

    with tile.TileContext(nc) as tc, ExitStack() as ctx:
        const_p = ctx.enter_context(tc.tile_pool(name="const", bufs=1))
        xm_p = ctx.enter_context(tc.tile_pool(name="xm", bufs=2))
        xs_p = ctx.enter_context(tc.tile_pool(name="xs", bufs=2))
        sp_p = ctx.enter_context(tc.tile_pool(name="spsum", bufs=4, space="PSUM"))
        ssb_p = ctx.enter_context(tc.tile_pool(name="ssb", bufs=2))
        m8_p = ctx.enter_context(tc.tile_pool(name="m8", bufs=2))
        i8_p = ctx.enter_context(tc.tile_pool(name="i8", bufs=2))
        stg_p = ctx.enter_context(tc.tile_pool(name="stg", bufs=2))
        stgd_p = ctx.enter_context(tc.tile_pool(name="stgd", bufs=2, space="DRAM"))
        ng_p = ctx.enter_context(tc.tile_pool(name="neigh", bufs=2))
        cp_p = ctx.enter_context(tc.tile_pool(name="cpsum", bufs=2, space="PSUM"))
        osb_p = ctx.enter_context(tc.tile_pool(name="osb", bufs=2))

        wt_sb = const_p.tile([C, K * O], f32)
        nc.sync.dma_start(out=wt_sb[:], in_=wt.rearrange("k c o -> c k o"))
        bias_sb = const_p.tile([O, 1], f32)
        nc.sync.dma_start(out=bias_sb[:], in_=bias)

        def body():
            emit_batches(nc, tc, x2, mn, ones, out, wt_sb, bias_sb, pools)

        pools = dict(
            xm_p=xm_p,
            xs_p=xs_p,
            sp_p=sp_p,
            ssb_p=ssb_p,
            m8_p=m8_p,
            i8_p=i8_p,
            stg_p=stg_p,
            stgd_p=stgd_p,
            ng_p=ng_p,
            cp_p=cp_p,
            osb_p=osb_p,
        )
        if reps > 1:
            with tc.For_i(0, reps, 1):
                body()
        else:
            body()

    nc.compile()
    return nc


def emit_batches(nc, tc, x2, mn, ones, out, wt_sb, bias_sb, pools):
    xm_p = pools["xm_p"]
    xs_p = pools["xs_p"]
    sp_p = pools["sp_p"]
    ssb_p = pools["ssb_p"]
    m8_p = pools["m8_p"]
    i8_p = pools["i8_p"]
    stg_p = pools["stg_p"]
    stgd_p = pools["stgd_p"]
    ng_p = pools["ng_p"]
    cp_p = pools["cp_p"]
    osb_p = pools["osb_p"]
    dist_dt = x2.dtype
    if True:
        for i in range(BPC):
            # column-chunked loads so the first matmul starts after ~1/4 of
            # the transfer instead of the whole 512 KB
            xm = xm_p.tile([C + 1, N], dist_dt)  # rows 0..63 = x, row 64 = mn
            xs = xs_p.tile([C + 1, N], dist_dt)  # rows 0..63 = x, row 64 = ones
            for c4 in range(N // FCH):
                cs = slice(c4 * FCH, (c4 + 1) * FCH)
                nc.sync.dma_start(out=xs[0:C, cs], in_=x2[i][:, cs])
                nc.sync.dma_start(out=xm[0:C, cs], in_=x2[i][:, cs])
            nc.sync.dma_start(out=xm[C : C + 1, :], in_=mn[i])
            nc.sync.dma_start(out=xs[C : C + 1, :], in_=ones)

            ob = osb_p.tile([O, N], f32)
            for t in range(NT):
                lhsT = xs[:, t * 128 : (t + 1) * 128]
                s_sb = ssb_p.tile([128, N], f32)
                # one PSUM tile per bank-sized chunk: deps stay per-chunk, so
                # ACT drains chunk c4 while the PE fills chunk c4+1.
                for c4 in range(N // FCH):
                    s = sp_p.tile([128, FCH], f32, tag="schunk")
                    nc.tensor.matmul(
                        s[:],
                        lhsT=lhsT,
                        rhs=xm[:, c4 * FCH : (c4 + 1) * FCH],
                        start=True,
                        stop=True,
                    )
                    nc.scalar.copy(
                        s_sb[:, c4 * FCH : (c4 + 1) * FCH],
                        s[:],
                    )
                m8 = m8_p.tile([128, 8], f32)
                nc.vector.max(m8[:], s_sb[:])
                i8 = i8_p.tile([128, 8], u16)
                nc.vector.max_index(i8[:], m8[:], s_sb[:])

                # this tile's 128x3 indices -> DRAM in the gather's wrapped
                # layout for a 384-index list (i = k*128 + r):
                # [p = r%16, slot = k*8 + r//16]
                sd = stgd_p.tile([16, K * 8], i16)
                sd_w = sd[:].rearrange("p (k a) -> a p k", k=K, a=8)
                nc.sync.dma_start(out=sd_w, in_=i8[:, 0:K].bitcast(i16))
                stg = stg_p.tile([C, K * 8], i16)
                # one DMA, 0-stride source dim replicates into all 4 Q7
                # core groups (each gpsimd core reads its own 16 partitions)
                nc.sync.dma_start(
                    out=stg[:],
                    in_=sd[:].rearrange("p f -> () p f").to_broadcast([4, 16, K * 8]),
                )

                # gather neighbors for these 128 columns (k-major), then the
                # 3-tap conv accumulates into one small psum chunk.
                ng = ng_p.tile([C, K * 128], f32)
                nc.gpsimd.ap_gather(
                    ng[:],
                    xm[0:C, :].bitcast(f32),
                    stg[:],
                    channels=C,
                    num_elems=N,
                    d=1,
                    num_idxs=K * 128,
                )
                cp = cp_p.tile([O, 128], f32)
                for k in range(K):
                    nc.tensor.matmul(
                        cp[:],
                        lhsT=wt_sb[:, k * O : (k + 1) * O],
                        rhs=ng[:, k * 128 : (k + 1) * 128],
                        start=(k == 0),
                        stop=(k == K - 1),
                    )
                nc.scalar.activation(
                    ob[:, t * 128 : (t + 1) * 128],
                    cp[:],
                    mybir.ActivationFunctionType.Identity,
                    bias=bias_sb[:],
                )
                if t % 4 == 3:  # stream results out per 512-column group
                    os = slice((t - 3) * 128, (t + 1) * 128)
                    nc.sync.dma_start(out=out[i][:, os], in_=ob[:, os])


_NC_CACHE: dict = {}


def _get_nc():
    if "nc" not in _NC_CACHE:
        _NC_CACHE["nc"] = build()
    return _NC_CACHE["nc"]


def make_in_maps(x, W, b):
    x = np.ascontiguousarray(np.asarray(x, dtype=np.float32))
    W = np.ascontiguousarray(np.asarray(W, dtype=np.float32))
    b = np.ascontiguousarray(np.asarray(b, dtype=np.float32))
    mn_full = -0.5 * np.einsum("bcn,bcn->bn", x, x)  # [B, N]
    wt = np.ascontiguousarray(np.transpose(W, (2, 1, 0)))  # [K, C, O]
    bias = np.ascontiguousarray(b.reshape(O, 1))
    ones = np.ones((1, N), dtype=np.float32)
    in_maps = []
    for c in range(NCORES):
        sl = slice(c * BPC, (c + 1) * BPC)
        in_maps.append(
            {
                "x2": np.ascontiguousarray(x[sl]),
                "mn": np.ascontiguousarray(mn_full[sl][:, None, :]),
                "ones": ones,
                "wt": wt,
                "bias": bias,
            }
        )
    return in_maps


def kernel(x, W, b, _trace=False):
    nc = _get_nc()
    in_maps = make_in_maps(x, W, b)
    try:
        res = run_bass_kernel_spmd(nc, in_maps, list(range(NCORES)), trace=_trace)
    except ModuleNotFoundError:
        res = run_bass_kernel_spmd(nc, in_maps, list(range(NCORES)))
    outs = [res.results[c]["out"] for c in range(NCORES)]
    full = np.concatenate(outs, axis=0).astype(np.float32)
    if _trace:
        return full, res
    return full
